# revision 33
# baseline (speedup 1.0000x reference)
"""Trainium2 Bass kernel for the 4-branch "Memory multimode" attention module.

Reference computation (per batch element b):
    q  = q_key[b].reshape(1024, 128)        (row-major reinterpret)
    pq = p_q_key[b].reshape(1024, 128)
    k  = m_key[b].reshape(128, 2048)
    pk = p_m_key[b].reshape(128, 2048)
    mval = m_val[b].reshape(512, 2048).T    # [2048, 512]
    out  = (sm(q@k) + sm(pq@pk) + sm(pq@k) + sm(q@pk)) @ mval
    where sm() is softmax over the QUERY dim (axis 0 of each [1024, 2048] score
    matrix).  Final output channel-concats q_val.

All four branches share the same value matrix, so the four softmax matrices
are summed BEFORE the value matmul - one [1024,2048]@[2048,512] matmul.

Design (one NeuronCore per batch element, 8 cores, data-parallel).
Measured engine busy (HW trace): ACT ~79us (the exp roofline - the pacer:
64 x [128x1024] exp+accum at ~1.24us each), PE ~76us (inflated by p-state
ramping; ~55us of work), DVE ~65us.  HW exec ~119us = ~10us fill +
~86us ACT-paced phase 1 + ~15us value-matmul tail + drain.
  * Scores in f32r single-pass (free dim 512 >= 256 -> 1 cyc/row, same PE
    speed as bf16 but ~2^-11 operand rounding; measured end-to-end rel err
    6e-4 << 2e-2 gate).  3x less PE time than the old bf16 hi/lo 3-pass.
  * E^T = exp(S^T) emitted as bf16: enables DVE 2x/4x perf modes downstream.
    Softmax denominators per (branch, key-position l) via accum_out (d_on=
    "act") or via a DVE tensor_scalar accum pass (d_on="dve", no 187ns/instr
    ACT accumulator-read penalty; DVE has no accum read charge).
  * Branch combine A^T = sum_br invd_br * E_br as tensor_scalar_mul (4x_2p
    on bf16/SBUF) + tensor_tensor adds (2x_1p) - the old scalar_tensor_tensor
    chain has NO DVE perf modes and ran at fp32 rate (77us).
  * Value matmul in bf16 (A^T bf16 from the combine, mval bf16): 1 cyc/row.
  * PSUM: 2x[128,1024] score tiles (4 banks) + n_overlap=4 output-row
    accumulators (4 banks); rows 4-7 drain in a short PE tail.
  * Output rows copied PSUM->SBUF on the (otherwise idle) Pool engine.
  * q_val never touches the device: concatenated on the host.
"""

import numpy as np
import ml_dtypes

import concourse.bass as bass
import concourse.mybir as mybir
import concourse.tile as tile
from concourse.bass_utils import run_bass_kernel_spmd
from concourse.vector_clock import ScopedClock

# The walrus build in this image supports only ONE sync-wait command per
# instruction (CTRL_NO_STRUCT / S3_LW_STRUCT encodings); this concourse's Tile
# scheduler freely attaches several.  Two fixes: (1) split the kernel-tail
# drain's waits over several drains, (2) a post-scheduling pass that moves
# overflow waits onto NoOps inserted before the over-subscribed instruction.
_MAX_WAITS = 1


def _split_drain_and_barrier(self, tick_clock, wait_clock):
    nc = self.nc
    drain_inst = nc.sync.drain()
    wait_clock.add_sem_waits(
        drain_inst.ins, ScopedClock({None: tick_clock.global_clock})
    )
    mi = drain_inst.ins
    waits = list(mi.sync_info.on_wait)
    if len(waits) > _MAX_WAITS:
        del mi.sync_info.on_wait[_MAX_WAITS:]
        rest = waits[_MAX_WAITS:]
        # distribute the singleton-wait drains across all engine sequencers
        # (they'd otherwise serialize ~50ns apiece on SP); the barrier below
        # joins them
        engs = [nc.sync, nc.scalar, nc.vector, nc.tensor, nc.gpsimd]
        for j, i in enumerate(range(0, len(rest), _MAX_WAITS)):
            extra = engs[j % len(engs)].drain()
            if extra.ins.sync_info is None:
                extra.ins.sync_info = mybir.SyncInfo(on_wait=[], on_update=[])
            extra.ins.sync_info.on_wait.extend(rest[i : i + _MAX_WAITS])

    nc.all_engine_barrier()
    assert self.sems is not None
    popped = nc._tile_sem_poison_stack.pop()
    assert popped is self._sem_poison
    nc.clear_and_free_semaphores(list(self.sems.allocated().values()))
    nc.all_engine_barrier()


tile.TileContext._drain_and_barrier = _split_drain_and_barrier


def _split_sync_waits(nc, cap: int = _MAX_WAITS):
    for f in nc.m.functions:
        for blk in f.blocks:
            out = []
            changed = False
            for inst in blk.instructions:
                si = inst.sync_info
                if si is not None and len(si.on_wait) > cap:
                    waits = list(si.on_wait)
                    rest, keep = waits[:-cap], waits[-cap:]
                    for i in range(0, len(rest), cap):
                        noop = mybir.InstNoOp(
                            name=nc.get_next_instruction_name(), ins=[], outs=[]
                        )
                        noop.engine = inst.engine
                        noop.sync_info = mybir.SyncInfo(
                            on_wait=rest[i : i + cap], on_update=[]
                        )
                        nc.register_instruction(noop)
                        out.append(noop)
                    inst.sync_info = mybir.SyncInfo(
                        on_wait=keep, on_update=list(si.on_update)
                    )
                    changed = True
                out.append(inst)
            if changed:
                blk.instructions = out
    return nc


B, H, W = 8, 32, 32
HW = H * W          # 1024 queries
KD = 128            # key dim
VD = 512            # val dim
L = 2 * HW          # 2048 key positions per key matrix
NT = L // 128       # 16 l-tiles
NCORES = 8

F32 = mybir.dt.float32
F32R = mybir.dt.float32r
BF16 = mybir.dt.bfloat16

_nc_cache = {}


def build_nc(n_overlap: int = 4, d_on: str = "act"):
    """d_on: where softmax denominators are reduced - 'act' (activation
    accum_out; +187ns/instr on the bottleneck ACT engine) or 'dve'
    (tensor_scalar accum pass on DVE at 4x bf16 rate)."""
    nc = bass.Bass("TRN2", target_bir_lowering=False, debug=False)

    def din(name, shape, dt):
        return nc.dram_tensor(name, shape, dt, kind="ExternalInput").ap()

    mk = din("mk", [KD, L], F32R)
    pmk = din("pmk", [KD, L], F32R)
    qt = din("qt", [KD, HW], F32R)
    pqt = din("pqt", [KD, HW], F32R)
    mvt = din("mvt", [L, VD], BF16)
    out = nc.dram_tensor("out", [HW, VD], F32, kind="ExternalOutput").ap()

    EXP = mybir.ActivationFunctionType.Exp
    MUL = mybir.AluOpType.mult
    NO = HW // 128  # 8 output row-tiles

    with tile.TileContext(nc) as tc:
        with (
            tc.tile_pool(name="keys", bufs=1) as keys_pool,
            tc.tile_pool(name="qts", bufs=1) as qt_pool,
            tc.tile_pool(name="mv", bufs=1) as mv_pool,
            tc.tile_pool(name="ework", bufs=3) as e_pool,
            tc.tile_pool(name="atiles", bufs=1) as a_pool,
            tc.tile_pool(name="dwork", bufs=4) as d_pool,
            tc.tile_pool(name="mwork", bufs=8) as m_pool,
            tc.tile_pool(name="ostage", bufs=8) as out_pool,
            tc.tile_pool(name="psum_s", bufs=2, space="PSUM") as psum_s,
            tc.tile_pool(name="psum_o", bufs=1, space="PSUM") as psum_o,
        ):
            # ---- input loads.  Two independent HW DGE rings: SP (nc.sync)
            # carries queries + values, ACT (nc.scalar, idle until the first
            # exp) carries keys.  Within each ring, first-needed chunks first:
            # the first score matmul needs qt[:, :1024] and keys[:, t0 tile].
            keys = keys_pool.tile([128, 2 * L], F32R, tag="k")
            qts = qt_pool.tile([128, 2 * HW], F32R, tag="q")
            # queries are needed in full from tile 0 (qt for xh=0, pqt for
            # xh=1); keys arrive incrementally per l-tile; mv tiles are first
            # touched by the value matmul of the corresponding l-tile.
            # (All on the SP ring: issuing DMAs from ACT delays its first
            # exp by ~3us.)
            nc.sync.dma_start(qts[:, 0:512], qt[:, 0:512])
            nc.sync.dma_start(keys[:, 0:128], mk[:, 0:128])
            nc.sync.dma_start(qts[:, 512:1024], qt[:, 512:1024])
            nc.sync.dma_start(keys[:, 128:512], mk[:, 128:512])
            for c in range(2):
                sl = slice(c * 512, (c + 1) * 512)
                nc.sync.dma_start(qts[:, HW + c * 512 : HW + (c + 1) * 512],
                                  pqt[:, sl])
            nc.sync.dma_start(keys[:, L : L + 512], pmk[:, 0:512])

            mv_tiles = [
                mv_pool.tile([128, VD], BF16, tag=f"mv{t}", name=f"mv{t}")
                for t in range(NT)
            ]
            mv_order = {1: [0, 1, 2, 3], 2: [4, 5, 6, 7, 8]}
            for qtr in range(1, 4):
                sl_d = slice(qtr * 512, (qtr + 1) * 512)
                for y, src in enumerate((mk, pmk)):
                    nc.sync.dma_start(keys[:, y * L + qtr * 512 :
                                           y * L + (qtr + 1) * 512],
                                      src[:, sl_d])
                for t in mv_order.get(qtr, range(9, NT)):
                    nc.sync.dma_start(mv_tiles[t][:],
                                      mvt[t * 128 : (t + 1) * 128, :])

            # phase-1-resident output accumulators (one PSUM bank each)
            o_acc = [
                psum_o.tile([128, VD], F32, tag=f"O{i}", name=f"o_acc{i}")
                for i in range(n_overlap)
            ]

            # ---- phase 1 ---------------------------------------------------
            # Value matmuls for tile t-1 are emitted BETWEEN the two score
            # groups of tile t: they only become runnable after combine(t-1),
            # and placing them behind scores(t, y0) in the PE stream keeps the
            # next EXP's operands the PE's first priority.
            a_tiles = []
            pending_value = []  # (a_sb, t) awaiting o_acc accumulation

            def emit_value(a_sb, t):
                for i in range(n_overlap):
                    nc.tensor.matmul(
                        o_acc[i][:],
                        a_sb[:, i * 128 : (i + 1) * 128],
                        mv_tiles[t][:],
                        start=(t == 0),
                        stop=(t == NT - 1),
                    )

            for t in range(NT):
                dtile = d_pool.tile([128, 4], F32, tag="D")
                e_tiles = []
                for y in range(2):
                    for xh in range(2):
                        # each quarter gets its own E tile: activations
                        # writing at offset 0 are ~110ns faster than at
                        # offset 2KB into the partition
                        e_q = e_pool.tile([128, HW], BF16, tag=f"E{y}{xh}",
                                          name=f"e_{t}_{y}_{xh}")
                        kslice = slice(y * L + t * 128, y * L + (t + 1) * 128)
                        s_ps = psum_s.tile([128, HW], F32, tag="S")
                        for c in range(2):
                            qslice = slice(xh * HW + c * 512, xh * HW + (c + 1) * 512)
                            nc.tensor.matmul(
                                s_ps[:, c * 512 : (c + 1) * 512],
                                keys[:, kslice], qts[:, qslice],
                                start=True, stop=True)
                        # E^T = exp(S^T); denom = row sum over queries
                        dcol = dtile[:, 2 * y + xh : 2 * y + xh + 1]
                        if d_on == "act":
                            nc.scalar.activation(e_q[:], s_ps[:], EXP,
                                                 accum_out=dcol)
                        else:
                            nc.scalar.activation(e_q[:], s_ps[:], EXP)
                            scr = m_pool.tile([128, HW], BF16, tag="dscr",
                                              name=f"dscr{t}_{y}_{xh}")
                            nc.vector.tensor_scalar(
                                scr[:], e_q[:], 1.0, 0.0, MUL,
                                mybir.AluOpType.add, accum_out=dcol)
                        e_tiles.append(e_q)
                    if y == 0 and pending_value:
                        emit_value(*pending_value.pop())

                invd = d_pool.tile([128, 4], F32, tag="invD")
                nc.vector.reciprocal(invd[:], dtile[:])

                # A^T[t] = sum_{y,xh} invD * E-half
                # tensor_scalar_mul (4x_2p @ bf16) + tensor_tensor add (2x_1p)
                a_sb = a_pool.tile([128, HW], BF16, tag=f"A{t}")
                mm = [m_pool.tile([128, HW], BF16, tag="m",
                                  name=f"m{j}_{t}") for j in range(4)]
                aa = [m_pool.tile([128, HW], BF16, tag="aa",
                                  name=f"aa{j}_{t}") for j in range(2)]
                for j in range(4):
                    nc.vector.tensor_scalar_mul(
                        mm[j][:], e_tiles[j][:], invd[:, j : j + 1])
                nc.vector.tensor_add(aa[0][:], mm[0][:], mm[1][:])
                nc.vector.tensor_add(aa[1][:], mm[2][:], mm[3][:])
                nc.vector.tensor_add(a_sb[:], aa[0][:], aa[1][:])
                a_tiles.append(a_sb)
                pending_value.append((a_sb, t))
            emit_value(*pending_value.pop())

            # ---- phase 2: drain overlapped rows, then the remaining rows.
            # Tail rows run in interleaved pairs (independent PSUM banks keep
            # the PE pipeline full); output DMAs alternate between the SP and
            # ACT rings (both idle by now) so the last rows drain in parallel.
            def drain(i, o_ps):
                o_sb = out_pool.tile([128, VD], F32, tag="osb",
                                     name=f"osb{i}")
                if i >= NO - 2:
                    # last rows: copy AND issue on ACT (idle by now) - same-
                    # engine ordering skips a cross-engine semaphore hop on
                    # the critical exit path
                    nc.scalar.copy(o_sb[:], o_ps[:])
                    nc.scalar.dma_start(out[i * 128 : (i + 1) * 128, :],
                                        o_sb[:])
                else:
                    # DVE copy: Pool can't touch PSUM, ACT is the bottleneck
                    nc.vector.tensor_copy(o_sb[:], o_ps[:])
                    eng = nc.sync if i % 2 == 0 else nc.scalar
                    eng.dma_start(out[i * 128 : (i + 1) * 128, :], o_sb[:])

            for i in range(n_overlap):
                drain(i, o_acc[i])
            for i in range(n_overlap, NO):
                o_ps = psum_s.tile([128, VD], F32, tag="S", name=f"o_tail{i}")
                for t in range(NT):
                    nc.tensor.matmul(
                        o_ps[:],
                        a_tiles[t][:, i * 128 : (i + 1) * 128],
                        mv_tiles[t][:],
                        start=(t == 0),
                        stop=(t == NT - 1),
                    )
                drain(i, o_ps)

    _split_sync_waits(nc)
    return nc


def make_in_maps(m_key, m_val, q_key, p_m_key, p_q_key):
    in_maps = []
    for b in range(B):
        m = {
            "mk": np.ascontiguousarray(m_key[b].reshape(KD, L)),
            "pmk": np.ascontiguousarray(p_m_key[b].reshape(KD, L)),
            "qt": np.ascontiguousarray(q_key[b].reshape(HW, KD).T),
            "pqt": np.ascontiguousarray(p_q_key[b].reshape(HW, KD).T),
            "mvt": np.ascontiguousarray(
                m_val[b].reshape(VD, L).T.astype(ml_dtypes.bfloat16)),
        }
        in_maps.append(m)
    return in_maps


def run(inputs, trace: bool = False, n_overlap: int = 4, d_on: str = "act"):
    """Run on the 8 NeuronCores; returns (full_output, BassKernelResults)."""
    inputs = {k: np.asarray(v, dtype=np.float32) for k, v in inputs.items()}
    key = (n_overlap, d_on)
    if key not in _nc_cache:
        _nc_cache[key] = build_nc(n_overlap, d_on)
    nc = _nc_cache[key]
    in_maps = make_in_maps(
        inputs["m_key"], inputs["m_val"], inputs["q_key"],
        inputs["p_m_key"], inputs["p_q_key"],
    )
    res = run_bass_kernel_spmd(nc, in_maps, list(range(NCORES)), trace=trace)
    q_val = inputs["q_val"]
    outs = []
    for b in range(B):
        mat = np.asarray(res.results[b]["out"])      # [1024, 512] row-major
        attn = mat.reshape(VD, H, W)                 # reinterpret, no transpose
        outs.append(np.concatenate([attn, q_val[b]], axis=0))
    return np.stack(outs), res


def kernel(**inputs) -> np.ndarray:
    out, _ = run(inputs, trace=False)
    return out


# revision 34
# speedup vs baseline: 1.1913x; 1.1913x over previous
"""Trainium2 Bass kernel for the 4-branch "Memory multimode" attention module.

Reference computation (per batch element b):
    q  = q_key[b].reshape(1024, 128)        (row-major reinterpret)
    pq = p_q_key[b].reshape(1024, 128)
    k  = m_key[b].reshape(128, 2048)
    pk = p_m_key[b].reshape(128, 2048)
    mval = m_val[b].reshape(512, 2048).T    # [2048, 512]
    out  = (sm(q@k) + sm(pq@pk) + sm(pq@k) + sm(q@pk)) @ mval
    where sm() is softmax over the QUERY dim (axis 0 of each [1024, 2048] score
    matrix).  Final output channel-concats q_val.

All four branches share the same value matrix, so the four softmax matrices
are summed BEFORE the value matmul - one [1024,2048]@[2048,512] matmul.

Design (one NeuronCore per batch element, 8 cores, data-parallel).
Measured engine busy (HW trace): ACT ~79us (the exp roofline - the pacer:
64 x [128x1024] exp+accum at ~1.24us each), PE ~76us (inflated by p-state
ramping; ~55us of work), DVE ~65us.  HW exec ~119us = ~10us fill +
~86us ACT-paced phase 1 + ~15us value-matmul tail + drain.
  * Scores in f32r single-pass (free dim 512 >= 256 -> 1 cyc/row, same PE
    speed as bf16 but ~2^-11 operand rounding; measured end-to-end rel err
    6e-4 << 2e-2 gate).  3x less PE time than the old bf16 hi/lo 3-pass.
  * E^T = exp(S^T) emitted as bf16: enables DVE 2x/4x perf modes downstream.
    Softmax denominators per (branch, key-position l) via accum_out (d_on=
    "act") or via a DVE tensor_scalar accum pass (d_on="dve", no 187ns/instr
    ACT accumulator-read penalty; DVE has no accum read charge).
  * Branch combine A^T = sum_br invd_br * E_br as tensor_scalar_mul (4x_2p
    on bf16/SBUF) + tensor_tensor adds (2x_1p) - the old scalar_tensor_tensor
    chain has NO DVE perf modes and ran at fp32 rate (77us).
  * Value matmul in bf16 (A^T bf16 from the combine, mval bf16): 1 cyc/row.
  * PSUM: 2x[128,1024] score tiles (4 banks) + n_overlap=4 output-row
    accumulators (4 banks); rows 4-7 drain in a short PE tail.
  * Output rows copied PSUM->SBUF on the (otherwise idle) Pool engine.
  * q_val never touches the device: concatenated on the host.
"""

import numpy as np
import ml_dtypes

import concourse.bass as bass
import concourse.mybir as mybir
import concourse.tile as tile
from concourse.bass_utils import run_bass_kernel_spmd
from concourse.vector_clock import ScopedClock

# The walrus build in this image supports only ONE sync-wait command per
# instruction (CTRL_NO_STRUCT / S3_LW_STRUCT encodings); this concourse's Tile
# scheduler freely attaches several.  Two fixes: (1) split the kernel-tail
# drain's waits over several drains, (2) a post-scheduling pass that moves
# overflow waits onto NoOps inserted before the over-subscribed instruction.
_MAX_WAITS = 1


def _split_drain_and_barrier(self, tick_clock, wait_clock):
    nc = self.nc
    drain_inst = nc.sync.drain()
    wait_clock.add_sem_waits(
        drain_inst.ins, ScopedClock({None: tick_clock.global_clock})
    )
    mi = drain_inst.ins
    waits = list(mi.sync_info.on_wait)
    if len(waits) > _MAX_WAITS:
        del mi.sync_info.on_wait[_MAX_WAITS:]
        rest = waits[_MAX_WAITS:]
        for i in range(0, len(rest), _MAX_WAITS):
            extra = nc.sync.drain()
            if extra.ins.sync_info is None:
                extra.ins.sync_info = mybir.SyncInfo(on_wait=[], on_update=[])
            extra.ins.sync_info.on_wait.extend(rest[i : i + _MAX_WAITS])

    nc.all_engine_barrier()
    assert self.sems is not None
    popped = nc._tile_sem_poison_stack.pop()
    assert popped is self._sem_poison
    nc.clear_and_free_semaphores(list(self.sems.allocated().values()))
    nc.all_engine_barrier()


tile.TileContext._drain_and_barrier = _split_drain_and_barrier


def _split_sync_waits(nc, cap: int = _MAX_WAITS):
    for f in nc.m.functions:
        for blk in f.blocks:
            out = []
            changed = False
            for inst in blk.instructions:
                si = inst.sync_info
                if si is not None and len(si.on_wait) > cap:
                    waits = list(si.on_wait)
                    rest, keep = waits[:-cap], waits[-cap:]
                    for i in range(0, len(rest), cap):
                        noop = mybir.InstNoOp(
                            name=nc.get_next_instruction_name(), ins=[], outs=[]
                        )
                        noop.engine = inst.engine
                        noop.sync_info = mybir.SyncInfo(
                            on_wait=rest[i : i + cap], on_update=[]
                        )
                        nc.register_instruction(noop)
                        out.append(noop)
                    inst.sync_info = mybir.SyncInfo(
                        on_wait=keep, on_update=list(si.on_update)
                    )
                    changed = True
                out.append(inst)
            if changed:
                blk.instructions = out
    return nc


B, H, W = 8, 32, 32
HW = H * W          # 1024 queries
KD = 128            # key dim
VD = 512            # val dim
L = 2 * HW          # 2048 key positions per key matrix
NT = L // 128       # 16 l-tiles
NCORES = 8

F32 = mybir.dt.float32
F32R = mybir.dt.float32r
BF16 = mybir.dt.bfloat16

_nc_cache = {}


def build_nc(n_overlap: int = 4, d_on: str = "act"):
    """d_on: where softmax denominators are reduced - 'act' (activation
    accum_out; +187ns/instr on the bottleneck ACT engine) or 'dve'
    (tensor_scalar accum pass on DVE at 4x bf16 rate)."""
    nc = bass.Bass("TRN2", target_bir_lowering=False, debug=False)

    def din(name, shape, dt):
        return nc.dram_tensor(name, shape, dt, kind="ExternalInput").ap()

    mk = din("mk", [KD, L], F32R)
    pmk = din("pmk", [KD, L], F32R)
    qt = din("qt", [KD, HW], F32R)
    pqt = din("pqt", [KD, HW], F32R)
    mvt = din("mvt", [L, VD], BF16)
    out = nc.dram_tensor("out", [HW, VD], F32, kind="ExternalOutput").ap()

    EXP = mybir.ActivationFunctionType.Exp
    MUL = mybir.AluOpType.mult
    NO = HW // 128  # 8 output row-tiles

    with tile.TileContext(nc) as tc:
        with (
            tc.tile_pool(name="keys", bufs=1) as keys_pool,
            tc.tile_pool(name="qts", bufs=1) as qt_pool,
            tc.tile_pool(name="mv", bufs=1) as mv_pool,
            tc.tile_pool(name="ework", bufs=3) as e_pool,
            tc.tile_pool(name="atiles", bufs=1) as a_pool,
            tc.tile_pool(name="dwork", bufs=4) as d_pool,
            tc.tile_pool(name="mwork", bufs=8) as m_pool,
            tc.tile_pool(name="ostage", bufs=8) as out_pool,
            tc.tile_pool(name="psum_s", bufs=2, space="PSUM") as psum_s,
            tc.tile_pool(name="psum_o", bufs=1, space="PSUM") as psum_o,
        ):
            # ---- input loads.  Two independent HW DGE rings: SP (nc.sync)
            # carries queries + values, ACT (nc.scalar, idle until the first
            # exp) carries keys.  Within each ring, first-needed chunks first:
            # the first score matmul needs qt[:, :1024] and keys[:, t0 tile].
            keys = keys_pool.tile([128, 2 * L], F32R, tag="k")
            qts = qt_pool.tile([128, 2 * HW], F32R, tag="q")
            # queries are needed in full from tile 0 (qt for xh=0, pqt for
            # xh=1); keys arrive incrementally per l-tile; mv tiles are first
            # touched by the value matmul of the corresponding l-tile.
            # (All on the SP ring: issuing DMAs from ACT delays its first
            # exp by ~3us.)
            nc.sync.dma_start(qts[:, 0:512], qt[:, 0:512])
            nc.sync.dma_start(keys[:, 0:128], mk[:, 0:128])
            nc.sync.dma_start(qts[:, 512:1024], qt[:, 512:1024])
            nc.sync.dma_start(keys[:, 128:512], mk[:, 128:512])
            for c in range(2):
                sl = slice(c * 512, (c + 1) * 512)
                nc.sync.dma_start(qts[:, HW + c * 512 : HW + (c + 1) * 512],
                                  pqt[:, sl])
            nc.sync.dma_start(keys[:, L : L + 512], pmk[:, 0:512])

            mv_tiles = [
                mv_pool.tile([128, VD], BF16, tag=f"mv{t}", name=f"mv{t}")
                for t in range(NT)
            ]
            mv_order = {1: [0, 1, 2, 3], 2: [4, 5, 6, 7, 8]}
            for qtr in range(1, 4):
                sl_d = slice(qtr * 512, (qtr + 1) * 512)
                for y, src in enumerate((mk, pmk)):
                    nc.sync.dma_start(keys[:, y * L + qtr * 512 :
                                           y * L + (qtr + 1) * 512],
                                      src[:, sl_d])
                for t in mv_order.get(qtr, range(9, NT)):
                    nc.sync.dma_start(mv_tiles[t][:],
                                      mvt[t * 128 : (t + 1) * 128, :])

            # phase-1-resident output accumulators (one PSUM bank each)
            o_acc = [
                psum_o.tile([128, VD], F32, tag=f"O{i}", name=f"o_acc{i}")
                for i in range(n_overlap)
            ]

            # ---- phase 1 ---------------------------------------------------
            # Value matmuls for tile t-1 are emitted BETWEEN the two score
            # groups of tile t: they only become runnable after combine(t-1),
            # and placing them behind scores(t, y0) in the PE stream keeps the
            # next EXP's operands the PE's first priority.
            a_tiles = []
            pending_value = []  # (a_sb, t) awaiting o_acc accumulation

            def emit_value(a_sb, t):
                for i in range(n_overlap):
                    nc.tensor.matmul(
                        o_acc[i][:],
                        a_sb[:, i * 128 : (i + 1) * 128],
                        mv_tiles[t][:],
                        start=(t == 0),
                        stop=(t == NT - 1),
                    )

            for t in range(NT):
                dtile = d_pool.tile([128, 4], F32, tag="D")
                e_tiles = []
                for y in range(2):
                    for xh in range(2):
                        # each quarter gets its own E tile: activations
                        # writing at offset 0 are ~110ns faster than at
                        # offset 2KB into the partition
                        e_q = e_pool.tile([128, HW], BF16, tag=f"E{y}{xh}",
                                          name=f"e_{t}_{y}_{xh}")
                        kslice = slice(y * L + t * 128, y * L + (t + 1) * 128)
                        s_ps = psum_s.tile([128, HW], F32, tag="S")
                        for c in range(2):
                            qslice = slice(xh * HW + c * 512, xh * HW + (c + 1) * 512)
                            nc.tensor.matmul(
                                s_ps[:, c * 512 : (c + 1) * 512],
                                keys[:, kslice], qts[:, qslice],
                                start=True, stop=True)
                        # E^T = exp(S^T); denom = row sum over queries
                        dcol = dtile[:, 2 * y + xh : 2 * y + xh + 1]
                        if d_on == "act":
                            nc.scalar.activation(e_q[:], s_ps[:], EXP,
                                                 accum_out=dcol)
                        else:
                            nc.scalar.activation(e_q[:], s_ps[:], EXP)
                            scr = m_pool.tile([128, HW], BF16, tag="dscr",
                                              name=f"dscr{t}_{y}_{xh}")
                            nc.vector.tensor_scalar(
                                scr[:], e_q[:], 1.0, 0.0, MUL,
                                mybir.AluOpType.add, accum_out=dcol)
                        e_tiles.append(e_q)
                    if y == 0 and pending_value:
                        emit_value(*pending_value.pop())

                invd = d_pool.tile([128, 4], F32, tag="invD")
                nc.vector.reciprocal(invd[:], dtile[:])

                # A^T[t] = sum_{y,xh} invD * E-half
                # tensor_scalar_mul (4x_2p @ bf16) + tensor_tensor add (2x_1p)
                a_sb = a_pool.tile([128, HW], BF16, tag=f"A{t}")
                mm = [m_pool.tile([128, HW], BF16, tag="m",
                                  name=f"m{j}_{t}") for j in range(4)]
                aa = [m_pool.tile([128, HW], BF16, tag="aa",
                                  name=f"aa{j}_{t}") for j in range(2)]
                for j in range(4):
                    nc.vector.tensor_scalar_mul(
                        mm[j][:], e_tiles[j][:], invd[:, j : j + 1])
                nc.vector.tensor_add(aa[0][:], mm[0][:], mm[1][:])
                nc.vector.tensor_add(aa[1][:], mm[2][:], mm[3][:])
                nc.vector.tensor_add(a_sb[:], aa[0][:], aa[1][:])
                a_tiles.append(a_sb)
                pending_value.append((a_sb, t))
            emit_value(*pending_value.pop())

            # ---- phase 2: drain overlapped rows, then the remaining rows.
            # Tail rows run in interleaved pairs (independent PSUM banks keep
            # the PE pipeline full); output DMAs alternate between the SP and
            # ACT rings (both idle by now) so the last rows drain in parallel.
            def drain(i, o_ps):
                o_sb = out_pool.tile([128, VD], F32, tag="osb",
                                     name=f"osb{i}")
                if i >= NO - 2:
                    # last rows: copy AND issue on ACT (idle by now) - same-
                    # engine ordering skips a cross-engine semaphore hop on
                    # the critical exit path
                    nc.scalar.copy(o_sb[:], o_ps[:])
                    nc.scalar.dma_start(out[i * 128 : (i + 1) * 128, :],
                                        o_sb[:])
                else:
                    # DVE copy: Pool can't touch PSUM, ACT is the bottleneck
                    nc.vector.tensor_copy(o_sb[:], o_ps[:])
                    eng = nc.sync if i % 2 == 0 else nc.scalar
                    eng.dma_start(out[i * 128 : (i + 1) * 128, :], o_sb[:])

            for i in range(n_overlap):
                drain(i, o_acc[i])
            for i in range(n_overlap, NO):
                o_ps = psum_s.tile([128, VD], F32, tag="S", name=f"o_tail{i}")
                for t in range(NT):
                    nc.tensor.matmul(
                        o_ps[:],
                        a_tiles[t][:, i * 128 : (i + 1) * 128],
                        mv_tiles[t][:],
                        start=(t == 0),
                        stop=(t == NT - 1),
                    )
                drain(i, o_ps)

    _split_sync_waits(nc)
    return nc


def make_in_maps(m_key, m_val, q_key, p_m_key, p_q_key):
    in_maps = []
    for b in range(B):
        m = {
            "mk": np.ascontiguousarray(m_key[b].reshape(KD, L)),
            "pmk": np.ascontiguousarray(p_m_key[b].reshape(KD, L)),
            "qt": np.ascontiguousarray(q_key[b].reshape(HW, KD).T),
            "pqt": np.ascontiguousarray(p_q_key[b].reshape(HW, KD).T),
            "mvt": np.ascontiguousarray(
                m_val[b].reshape(VD, L).T.astype(ml_dtypes.bfloat16)),
        }
        in_maps.append(m)
    return in_maps


def run(inputs, trace: bool = False, n_overlap: int = 4, d_on: str = "act"):
    """Run on the 8 NeuronCores; returns (full_output, BassKernelResults)."""
    inputs = {k: np.asarray(v, dtype=np.float32) for k, v in inputs.items()}
    key = (n_overlap, d_on)
    if key not in _nc_cache:
        _nc_cache[key] = build_nc(n_overlap, d_on)
    nc = _nc_cache[key]
    in_maps = make_in_maps(
        inputs["m_key"], inputs["m_val"], inputs["q_key"],
        inputs["p_m_key"], inputs["p_q_key"],
    )
    res = run_bass_kernel_spmd(nc, in_maps, list(range(NCORES)), trace=trace)
    q_val = inputs["q_val"]
    outs = []
    for b in range(B):
        mat = np.asarray(res.results[b]["out"])      # [1024, 512] row-major
        attn = mat.reshape(VD, H, W)                 # reinterpret, no transpose
        outs.append(np.concatenate([attn, q_val[b]], axis=0))
    return np.stack(outs), res


def kernel(**inputs) -> np.ndarray:
    out, _ = run(inputs, trace=False)
    return out
